# revision 32
# baseline (speedup 1.0000x reference)
"""KNN graph augmentation kernel for Trainium2 (8 NeuronCores, SPMD).

Problem: for 8 graphs of 4096 3-D points each, build the k=50 nearest
neighbor graph per graph (excluding self loops), then emit
  edge_index [2, 2*N*K]  (forward + reversed duplicate, int32)
  dist       [2*N*K, 1]  euclidean edge length (f32)
  edge_attr  [2*N*K, 5]  gaussian RDF expansion of dist (f32)

Sharding: data-parallel over graphs, one graph per NeuronCore.

Device algorithm per core (M=4096 nodes, 32 row-tiles of 128):
  - selection key s' = 2*p_i.p_j - |p_j|^2  (equals -d2 + |p_i|^2; the
    per-row constant does not affect per-row top-k order) via a K=4
    augmented fp32 matmul on the PE, evicted PSUM->SBUF by the scalar
    engine. Diagonal masked to -BIG with gpsimd affine_select.
  - top-50 per row ("opt" variant): exact top-16 of each 256-wide
    segment (DVE max8/find_index8/match_replace8, stage-batched), then
    a 256-candidate merge for the top-56 in order, then two gpsimd
    local_scatters invert rank->position into rank-ordered global
    column ids. A segment holding >16 of a row's top-50 would be the
    only error source; P ~ 3e-8 per segment-row (top-50 columns are
    uniform), ~0.02 expected rows over the whole problem.
  - dist = sqrt(relu(|p_i|^2 - s'_sel)), RDF bins on the scalar engine.
Host reassembles the full mirrored edge list (pure memcpy-class work).

Measured on trn2 (8 cores, SPMD): HW exec ~1.245 ms; vs jax CPU
reference L2 rel err ~5e-4 on edge_index (near-tie order flips only),
~1e-6 on dist/edge_attr.
"""

import numpy as np

N = 32768
G = 8
M = N // G          # 4096 nodes per graph/core
K = 50
NUM_BINS = 5
CUTOFF = 10.0
P = 128             # partitions
T = M // P          # 32 row tiles per core
CCH = 512           # matmul free-dim chunk
NCH = M // CCH      # 8 chunks
KR = 7              # ceil(K/8) rounds of 8-wide extraction
NEG_BIG = -3.0e38

# top-k variant: "safe" = 7x(max/max_index/match_replace) over full 4096 row;
# "opt" = per-256-segment top-16 + merge + gpsimd local_scatter index fixup
VARIANT = "opt"
NSEG = 16           # segments per row (opt variant)
W = M // NSEG       # 256 segment width

_CACHE = {}


def _build_bass(variant):
    import concourse.bacc as bacc
    import concourse.mybir as mybir
    from concourse.tile import TileContext
    from concourse.masks import make_identity
    from concourse.bass import ts
    from contextlib import ExitStack

    f32 = mybir.dt.float32
    u32 = mybir.dt.uint32
    u16 = mybir.dt.uint16
    i16 = mybir.dt.int16
    AF = mybir.ActivationFunctionType

    width = CUTOFF / (NUM_BINS - 1)          # 2.5
    inv2w2 = 1.0 / (2.0 * width * width)     # 0.08
    centers = [i * width for i in range(NUM_BINS)]

    nc = bacc.Bacc()
    pos_d = nc.dram_tensor("pos", [M, 3], f32, kind="ExternalInput")
    nbr_dt = u32 if variant == "safe" else u16
    nbr_d = nc.dram_tensor("nbr", [M, K], nbr_dt, kind="ExternalOutput")
    dist_d = nc.dram_tensor("dist", [M, K], f32, kind="ExternalOutput")
    attr_d = nc.dram_tensor("attr", [M, K * NUM_BINS], f32, kind="ExternalOutput")

    with ExitStack() as ctx:
        tc = ctx.enter_context(TileContext(nc))
        const = ctx.enter_context(tc.tile_pool(name="const", bufs=1))
        setup = ctx.enter_context(tc.tile_pool(name="setup", bufs=3))
        setup_ps = ctx.enter_context(tc.tile_pool(name="setup_ps", bufs=3, space="PSUM"))
        sq_ps = ctx.enter_context(tc.tile_pool(name="sq_ps", bufs=1, space="PSUM"))
        mm_ps = ctx.enter_context(tc.tile_pool(name="mm_ps", bufs=4, space="PSUM"))
        s_pool = ctx.enter_context(tc.tile_pool(name="s_pool", bufs=4))
        sm_pool = ctx.enter_context(tc.tile_pool(name="sm_pool", bufs=6))
        out_pool = ctx.enter_context(tc.tile_pool(name="out_pool", bufs=6))

        identity = const.tile([P, P], f32)
        make_identity(nc, identity)

        lhsT = const.tile([4, M], f32)   # rows 0..2: p^T, row 3: ones
        rhs = const.tile([4, M], f32)    # rows 0..2: 2*p^T, row 3: -|p_j|^2
        ones3 = const.tile([3, 1], f32)
        sq_part = const.tile([P, T], f32)  # |p_i|^2 in row-tile layout
        # engine APs must start at partition 0: memset all 4 rows to 1.0,
        # rows 0..2 are overwritten by the transposes below; row 3 stays 1.
        nc.vector.memset(lhsT, 1.0)
        nc.vector.memset(ones3, 1.0)
        negc = const.tile([P, NUM_BINS], f32)  # per-bin bias = -center_b
        for b in range(NUM_BINS):
            nc.vector.memset(negc[:, b:b + 1], -centers[b])

        if variant == "opt":
            # candidate slot c = 16*seg + j  ->  global col = W*seg + local
            seg_off = const.tile([P, NSEG * 16], u16)
            nc.gpsimd.iota(seg_off, pattern=[[W, NSEG], [0, 16]], base=0,
                           channel_multiplier=0)
            ranks1 = const.tile([P, KR * 8], u16)    # 1..56
            nc.gpsimd.iota(ranks1, pattern=[[1, KR * 8]], base=1,
                           channel_multiplier=0)

        # --- setup: transpose pos into [3, M], accumulate |p|^2 ---
        for t in range(T):
            pos_t = setup.tile([P, 3], f32)
            nc.sync.dma_start(out=pos_t, in_=pos_d[t * P:(t + 1) * P, :])
            sq3 = setup.tile([P, 3], f32)
            nc.scalar.activation(sq3, pos_t, AF.Square,
                                 accum_out=sq_part[:, t:t + 1])
            ps_tr = setup_ps.tile([P, P], f32)
            nc.tensor.matmul(ps_tr[:3, :], pos_t, identity, is_transpose=True)
            nc.vector.tensor_copy(lhsT[0:3, ts(t, P)], ps_tr[:3, :])
            nc.scalar.activation(rhs[0:3, ts(t, P)], ps_tr[:3, :], AF.Copy,
                                 scale=2.0)

        # row 3 of rhs: -|p_j|^2 via ones.T @ (p^T * p^T), staged in a
        # partition-0 row then DMA'd to partition 3 (engine APs cannot
        # start at partition 3, DMA can).
        pT2 = const.tile([3, M], f32)
        negsq_row = const.tile([1, M], f32)
        for c in range(NCH):
            # per-chunk so chunk c only depends on transposes 4c..4c+3
            nc.vector.tensor_mul(pT2[:, ts(c, CCH)], lhsT[0:3, ts(c, CCH)],
                                 lhsT[0:3, ts(c, CCH)])
            ps_sq = sq_ps.tile([1, CCH], f32)
            nc.tensor.matmul(ps_sq, ones3, pT2[:, ts(c, CCH)])
            nc.scalar.activation(negsq_row[:, ts(c, CCH)], ps_sq, AF.Copy,
                                 scale=-1.0)
            nc.sync.dma_start(out=rhs[3:4, ts(c, CCH)],
                              in_=negsq_row[:, ts(c, CCH)])

        # --- main loop over 32 row tiles ---
        for t in range(T):
            s_tile = s_pool.tile([P, M], f32)
            for c in range(NCH):
                ps = mm_ps.tile([P, CCH], f32)
                nc.tensor.matmul(ps, lhsT[:, ts(t, P)], rhs[:, ts(c, CCH)],
                                 start=True, stop=True)
                nc.scalar.activation(s_tile[:, ts(c, CCH)], ps, AF.Copy)

            # mask self-distance: fill where (col - 128*t - p) == 0
            nc.gpsimd.affine_select(
                out=s_tile[:, ts(t, P)], in_=s_tile[:, ts(t, P)],
                compare_op=mybir.AluOpType.not_equal, fill=NEG_BIG,
                base=0, pattern=[[1, P]], channel_multiplier=-1,
            )

            if variant == "safe":
                vals = sm_pool.tile([P, KR * 8], f32)
                idxs = sm_pool.tile([P, KR * 8], u32)
                for r in range(KR):
                    nc.vector.max(out=vals[:, 8 * r:8 * r + 8], in_=s_tile)
                    nc.vector.max_index(out=idxs[:, 8 * r:8 * r + 8],
                                        in_max=vals[:, 8 * r:8 * r + 8],
                                        in_values=s_tile)
                    if r < KR - 1:
                        nc.vector.match_replace(
                            out=s_tile,
                            in_to_replace=vals[:, 8 * r:8 * r + 8],
                            in_values=s_tile, imm_value=NEG_BIG)
                nbr_sb = idxs
            else:
                # phase 1: top-16 of each 256-wide segment (values + local
                # idx). Emitted stage-batched: consecutive DVE ops are
                # independent (different segments), so the serial
                # max->max_index->match_replace latency is hidden by the
                # engine's in-order pipeline instead of paid per segment.
                cand_v = sm_pool.tile([P, NSEG * 16], f32)
                cand_li = sm_pool.tile([P, NSEG * 16], u16)

                def _seg(g):
                    return s_tile[:, g * W:(g + 1) * W]

                for g in range(NSEG):
                    nc.vector.max(out=cand_v[:, 16 * g:16 * g + 8],
                                  in_=_seg(g))
                for g in range(NSEG):
                    nc.vector.max_index(out=cand_li[:, 16 * g:16 * g + 8],
                                        in_max=cand_v[:, 16 * g:16 * g + 8],
                                        in_values=_seg(g))
                for g in range(NSEG):
                    nc.vector.match_replace(
                        out=_seg(g),
                        in_to_replace=cand_v[:, 16 * g:16 * g + 8],
                        in_values=_seg(g), imm_value=NEG_BIG)
                for g in range(NSEG):
                    nc.vector.max(out=cand_v[:, 16 * g + 8:16 * g + 16],
                                  in_=_seg(g))
                for g in range(NSEG):
                    nc.vector.max_index(
                        out=cand_li[:, 16 * g + 8:16 * g + 16],
                        in_max=cand_v[:, 16 * g + 8:16 * g + 16],
                        in_values=_seg(g))
                # global column id per candidate (u16 int ops: DVE only)
                cand_gi = sm_pool.tile([P, NSEG * 16], u16)
                nc.vector.tensor_add(cand_gi, cand_li, seg_off)

                # phase 2: merge 256 candidates -> top-56 values + positions.
                # The max/match_replace alternation is inherently serial, but
                # the position lookups are not: run all 7 find_index8 as an
                # independent batch against a pristine copy so their search
                # latency pipelines (~195ns vs ~560ns serial).
                cand_v0 = sm_pool.tile([P, NSEG * 16], f32)
                nc.vector.tensor_copy(cand_v0, cand_v)
                vals = sm_pool.tile([P, KR * 8], f32)
                wpos = sm_pool.tile([P, KR * 8], u16)
                for r in range(KR):
                    nc.vector.max(out=vals[:, 8 * r:8 * r + 8], in_=cand_v)
                    if r < KR - 1:
                        nc.vector.match_replace(
                            out=cand_v,
                            in_to_replace=vals[:, 8 * r:8 * r + 8],
                            in_values=cand_v, imm_value=NEG_BIG)
                for r in range(KR):
                    nc.vector.max_index(out=wpos[:, 8 * r:8 * r + 8],
                                        in_max=vals[:, 8 * r:8 * r + 8],
                                        in_values=cand_v0)

                # phase 3 (gpsimd): invert rank->pos into ordered global ids.
                # rank_at[pos] = rank+1 (0 for non-winners); then scatter all
                # candidates to slot rank_at[c] -- non-winners pile up on
                # slot 0 (last-write-wins, verified on HW), winners land on
                # slots 1..56 in rank order.
                rank_at = sm_pool.tile([P, NSEG * 16], u16)
                nc.gpsimd.local_scatter(
                    out_ap=rank_at[:, :], data_ap=ranks1[:, :],
                    idxs_ap=wpos[:, :].bitcast(i16),
                    channels=P, num_elems=NSEG * 16, num_idxs=KR * 8)
                ordered = sm_pool.tile([P, 64], u16)
                nc.gpsimd.local_scatter(
                    out_ap=ordered[:, :], data_ap=cand_gi[:, :],
                    idxs_ap=rank_at[:, :].bitcast(i16),
                    channels=P, num_elems=64, num_idxs=NSEG * 16)
                nbr_sb = ordered[:, 1:]

            # dist = sqrt(relu(|p_i|^2 - s'))
            d2t = sm_pool.tile([P, K], f32)
            nc.scalar.activation(d2t, vals[:, :K], AF.Relu,
                                 bias=sq_part[:, t:t + 1], scale=-1.0)
            dist_t = out_pool.tile([P, K], f32)
            nc.scalar.activation(dist_t, d2t, AF.Sqrt)

            # RDF bins: exp(-(dist - c_b)^2 / (2 w^2))
            attr_t = out_pool.tile([P, K, NUM_BINS], f32)
            for b in range(NUM_BINS):
                u = sm_pool.tile([P, K], f32, tag="u_tmp")
                nc.scalar.activation(u, dist_t, AF.Square,
                                     bias=negc[:, b:b + 1])
                nc.scalar.activation(attr_t[:, :, b], u, AF.Exp,
                                     scale=-inv2w2)

            nc.sync.dma_start(out=nbr_d[ts(t, P), :], in_=nbr_sb[:, :K])
            nc.sync.dma_start(out=dist_d[ts(t, P), :], in_=dist_t)
            nc.sync.dma_start(
                out=attr_d[ts(t, P), :],
                in_=attr_t[:].rearrange("p a b -> p (a b)"),
            )

    if not nc.is_finalized():
        nc.finalize()
    return nc


def _get_nc():
    key = "nc_" + VARIANT
    if key not in _CACHE:
        _CACHE[key] = _build_bass(VARIANT)
    return _CACHE[key]


def run_device(pos_full, trace=False):
    """Run the SPMD kernel. Returns (per_core_results, BassKernelResults)."""
    from concourse.bass_utils import run_bass_kernel_spmd

    pos_full = np.ascontiguousarray(np.asarray(pos_full, dtype=np.float32))
    assert pos_full.shape == (N, 3)
    in_maps = [{"pos": pos_full[g * M:(g + 1) * M]} for g in range(G)]
    res = run_bass_kernel_spmd(_get_nc(), in_maps, list(range(G)), trace=trace)
    return res.results, res


_DST = None


def _static_dst():
    global _DST
    if _DST is None:
        _DST = np.repeat(np.arange(N, dtype=np.int32), K)
    return _DST


def assemble(results):
    nbr = np.stack([r["nbr"].astype(np.int32, copy=False) for r in results])
    dist = np.stack([r["dist"] for r in results])        # [G, M, K]
    attr = np.stack([r["attr"] for r in results])        # [G, M, K*5]

    offs = (np.arange(G, dtype=np.int32) * M)[:, None, None]
    src = (nbr + offs).reshape(-1)
    dst = _static_dst()

    E = N * K
    edge_index = np.empty((2, 2 * E), dtype=np.int32)
    edge_index[0, :E] = src
    edge_index[0, E:] = dst
    edge_index[1, :E] = dst
    edge_index[1, E:] = src

    d = dist.reshape(-1)
    dist_full = np.empty((2 * E, 1), dtype=np.float32)
    dist_full[:E, 0] = d
    dist_full[E:, 0] = d

    a = attr.reshape(-1, NUM_BINS)
    attr_full = np.empty((2 * E, NUM_BINS), dtype=np.float32)
    attr_full[:E] = a
    attr_full[E:] = a

    return edge_index, dist_full, attr_full


def _results_sane(results):
    """Cheap guard against rare transient device glitches (garbage runs)."""
    try:
        for r in results:
            if int(r["nbr"].max()) >= M:
                return False
            if not np.isfinite(r["dist"]).all():
                return False
    except Exception:
        return False
    return True


def kernel(pos, batch=None, num_graphs=None, **kw):
    results, _ = run_device(pos)
    if not _results_sane(results):
        results, _ = run_device(pos)
    return assemble(results)


# revision 38
# speedup vs baseline: 1.1243x; 1.1243x over previous
"""KNN graph augmentation kernel for Trainium2 (8 NeuronCores, SPMD).

Problem: for 8 graphs of 4096 3-D points each, build the k=50 nearest
neighbor graph per graph (excluding self loops), then emit
  edge_index [2, 2*N*K]  (forward + reversed duplicate, int32)
  dist       [2*N*K, 1]  euclidean edge length (f32)
  edge_attr  [2*N*K, 5]  gaussian RDF expansion of dist (f32)

Sharding: data-parallel over graphs, one graph per NeuronCore.

Device algorithm per core (M=4096 nodes, 32 row-tiles of 128):
  - selection key s' = 2*p_i.p_j - |p_j|^2  (equals -d2 + |p_i|^2; the
    per-row constant does not affect per-row top-k order) via a K=4
    augmented fp32 matmul on the PE, evicted PSUM->SBUF by the scalar
    engine. Diagonal masked to -BIG with gpsimd affine_select.
  - top-50 per row ("opt" variant): exact top-16 of each 256-wide
    segment (DVE max8/find_index8/match_replace8, stage-batched), then
    a 256-candidate merge for the top-56 in order, then two gpsimd
    local_scatters invert rank->position into rank-ordered global
    column ids. A segment holding >16 of a row's top-50 would be the
    only error source; P ~ 3e-8 per segment-row (top-50 columns are
    uniform), ~0.02 expected rows over the whole problem.
  - dist = sqrt(relu(|p_i|^2 - s'_sel)), RDF bins on the scalar engine.
Host reassembles the full mirrored edge list (pure memcpy-class work).

Measured on trn2 (8 cores, SPMD): HW exec ~1.245 ms; vs jax CPU
reference L2 rel err ~5e-4 on edge_index (near-tie order flips only),
~1e-6 on dist/edge_attr.
"""

import numpy as np

N = 32768
G = 8
M = N // G          # 4096 nodes per graph/core
K = 50
NUM_BINS = 5
CUTOFF = 10.0
P = 128             # partitions
T = M // P          # 32 row tiles per core
CCH = 512           # matmul free-dim chunk
NCH = M // CCH      # 8 chunks
KR = 7              # ceil(K/8) rounds of 8-wide extraction
NEG_BIG = -3.0e38

# top-k variant: "safe" = 7x(max/max_index/match_replace) over full 4096 row;
# "opt" = per-256-segment top-16 + merge + gpsimd local_scatter index fixup
VARIANT = "opt"
NSEG = 8            # segments per row (opt variant)
W = M // NSEG       # 512 segment width

_CACHE = {}


def _build_bass(variant):
    import concourse.bacc as bacc
    import concourse.mybir as mybir
    from concourse.tile import TileContext
    from concourse.masks import make_identity
    from concourse.bass import ts
    from contextlib import ExitStack

    f32 = mybir.dt.float32
    u32 = mybir.dt.uint32
    u16 = mybir.dt.uint16
    i16 = mybir.dt.int16
    AF = mybir.ActivationFunctionType

    width = CUTOFF / (NUM_BINS - 1)          # 2.5
    inv2w2 = 1.0 / (2.0 * width * width)     # 0.08
    centers = [i * width for i in range(NUM_BINS)]

    nc = bacc.Bacc()
    pos_d = nc.dram_tensor("pos", [M, 3], f32, kind="ExternalInput")
    nbr_dt = u32 if variant == "safe" else u16
    nbr_d = nc.dram_tensor("nbr", [M, K], nbr_dt, kind="ExternalOutput")
    dist_d = nc.dram_tensor("dist", [M, K], f32, kind="ExternalOutput")
    attr_d = nc.dram_tensor("attr", [M, K * NUM_BINS], f32, kind="ExternalOutput")
    if variant == "opt":
        # 16th-kept s' per segment: lets the host detect (rare) capacity
        # overflow of a segment and recompute those rows exactly.
        v16_d = nc.dram_tensor("v16", [M, NSEG], f32, kind="ExternalOutput")

    with ExitStack() as ctx:
        tc = ctx.enter_context(TileContext(nc))
        const = ctx.enter_context(tc.tile_pool(name="const", bufs=1))
        setup = ctx.enter_context(tc.tile_pool(name="setup", bufs=3))
        setup_ps = ctx.enter_context(tc.tile_pool(name="setup_ps", bufs=3, space="PSUM"))
        sq_ps = ctx.enter_context(tc.tile_pool(name="sq_ps", bufs=1, space="PSUM"))
        mm_ps = ctx.enter_context(tc.tile_pool(name="mm_ps", bufs=4, space="PSUM"))
        s_pool = ctx.enter_context(tc.tile_pool(name="s_pool", bufs=4))
        sm_pool = ctx.enter_context(tc.tile_pool(name="sm_pool", bufs=6))
        out_pool = ctx.enter_context(tc.tile_pool(name="out_pool", bufs=6))

        identity = const.tile([P, P], f32)
        make_identity(nc, identity)

        lhsT = const.tile([4, M], f32)   # rows 0..2: p^T, row 3: ones
        rhs = const.tile([4, M], f32)    # rows 0..2: 2*p^T, row 3: -|p_j|^2
        ones3 = const.tile([3, 1], f32)
        sq_part = const.tile([P, T], f32)  # |p_i|^2 in row-tile layout
        # engine APs must start at partition 0: memset all 4 rows to 1.0,
        # rows 0..2 are overwritten by the transposes below; row 3 stays 1.
        nc.vector.memset(lhsT, 1.0)
        nc.vector.memset(ones3, 1.0)
        negc = const.tile([P, NUM_BINS], f32)  # per-bin bias = -center_b
        for b in range(NUM_BINS):
            nc.vector.memset(negc[:, b:b + 1], -centers[b])

        if variant == "opt":
            # candidate slot c = 16*seg + j  ->  global col = W*seg + local
            seg_off = const.tile([P, NSEG * 16], u16)
            nc.gpsimd.iota(seg_off, pattern=[[W, NSEG], [0, 16]], base=0,
                           channel_multiplier=0)
            ranks1 = const.tile([P, KR * 8], u16)    # 1..56
            nc.gpsimd.iota(ranks1, pattern=[[1, KR * 8]], base=1,
                           channel_multiplier=0)

        # --- setup: transpose pos into [3, M], accumulate |p|^2 ---
        for t in range(T):
            pos_t = setup.tile([P, 3], f32)
            nc.sync.dma_start(out=pos_t, in_=pos_d[t * P:(t + 1) * P, :])
            sq3 = setup.tile([P, 3], f32)
            nc.scalar.activation(sq3, pos_t, AF.Square,
                                 accum_out=sq_part[:, t:t + 1])
            ps_tr = setup_ps.tile([P, P], f32)
            nc.tensor.matmul(ps_tr[:3, :], pos_t, identity, is_transpose=True)
            nc.vector.tensor_copy(lhsT[0:3, ts(t, P)], ps_tr[:3, :])
            nc.scalar.activation(rhs[0:3, ts(t, P)], ps_tr[:3, :], AF.Copy,
                                 scale=2.0)

        # row 3 of rhs: -|p_j|^2 via ones.T @ (p^T * p^T), staged in a
        # partition-0 row then DMA'd to partition 3 (engine APs cannot
        # start at partition 3, DMA can).
        pT2 = const.tile([3, M], f32)
        negsq_row = const.tile([1, M], f32)
        for c in range(NCH):
            # per-chunk so chunk c only depends on transposes 4c..4c+3
            nc.vector.tensor_mul(pT2[:, ts(c, CCH)], lhsT[0:3, ts(c, CCH)],
                                 lhsT[0:3, ts(c, CCH)])
            ps_sq = sq_ps.tile([1, CCH], f32)
            nc.tensor.matmul(ps_sq, ones3, pT2[:, ts(c, CCH)])
            nc.scalar.activation(negsq_row[:, ts(c, CCH)], ps_sq, AF.Copy,
                                 scale=-1.0)
            nc.sync.dma_start(out=rhs[3:4, ts(c, CCH)],
                              in_=negsq_row[:, ts(c, CCH)])

        # --- main loop over 32 row tiles ---
        for t in range(T):
            s_tile = s_pool.tile([P, M], f32)
            for c in range(NCH):
                ps = mm_ps.tile([P, CCH], f32)
                nc.tensor.matmul(ps, lhsT[:, ts(t, P)], rhs[:, ts(c, CCH)],
                                 start=True, stop=True)
                nc.scalar.activation(s_tile[:, ts(c, CCH)], ps, AF.Copy)

            # mask self-distance: fill where (col - 128*t - p) == 0
            nc.gpsimd.affine_select(
                out=s_tile[:, ts(t, P)], in_=s_tile[:, ts(t, P)],
                compare_op=mybir.AluOpType.not_equal, fill=NEG_BIG,
                base=0, pattern=[[1, P]], channel_multiplier=-1,
            )

            if variant == "safe":
                vals = sm_pool.tile([P, KR * 8], f32)
                idxs = sm_pool.tile([P, KR * 8], u32)
                for r in range(KR):
                    nc.vector.max(out=vals[:, 8 * r:8 * r + 8], in_=s_tile)
                    nc.vector.max_index(out=idxs[:, 8 * r:8 * r + 8],
                                        in_max=vals[:, 8 * r:8 * r + 8],
                                        in_values=s_tile)
                    if r < KR - 1:
                        nc.vector.match_replace(
                            out=s_tile,
                            in_to_replace=vals[:, 8 * r:8 * r + 8],
                            in_values=s_tile, imm_value=NEG_BIG)
                nbr_sb = idxs
            else:
                # phase 1: top-16 of each 256-wide segment (values + local
                # idx). Emitted stage-batched: consecutive DVE ops are
                # independent (different segments), so the serial
                # max->max_index->match_replace latency is hidden by the
                # engine's in-order pipeline instead of paid per segment.
                cand_v = sm_pool.tile([P, NSEG * 16], f32)
                cand_li = sm_pool.tile([P, NSEG * 16], u16)

                def _seg(g):
                    return s_tile[:, g * W:(g + 1) * W]

                for g in range(NSEG):
                    nc.vector.max(out=cand_v[:, 16 * g:16 * g + 8],
                                  in_=_seg(g))
                for g in range(NSEG):
                    nc.vector.max_index(out=cand_li[:, 16 * g:16 * g + 8],
                                        in_max=cand_v[:, 16 * g:16 * g + 8],
                                        in_values=_seg(g))
                for g in range(NSEG):
                    nc.vector.match_replace(
                        out=_seg(g),
                        in_to_replace=cand_v[:, 16 * g:16 * g + 8],
                        in_values=_seg(g), imm_value=NEG_BIG)
                for g in range(NSEG):
                    nc.vector.max(out=cand_v[:, 16 * g + 8:16 * g + 16],
                                  in_=_seg(g))
                for g in range(NSEG):
                    nc.vector.max_index(
                        out=cand_li[:, 16 * g + 8:16 * g + 16],
                        in_max=cand_v[:, 16 * g + 8:16 * g + 16],
                        in_values=_seg(g))
                # global column id per candidate (u16 int ops: DVE only)
                cand_gi = sm_pool.tile([P, NSEG * 16], u16)
                nc.vector.tensor_add(cand_gi, cand_li, seg_off)

                # export each segment's 16th-kept value (overflow detector);
                # DMA reads the strided slots before the merge overwrites them
                nc.sync.dma_start(
                    out=v16_d[ts(t, P), :],
                    in_=cand_v[:].rearrange("p (s c) -> p s c", c=16)[:, :, 15],
                )

                # phase 2: merge 256 candidates -> top-56 values + positions
                vals = sm_pool.tile([P, KR * 8], f32)
                wpos = sm_pool.tile([P, KR * 8], u16)
                for r in range(KR):
                    nc.vector.max(out=vals[:, 8 * r:8 * r + 8], in_=cand_v)
                    nc.vector.max_index(out=wpos[:, 8 * r:8 * r + 8],
                                        in_max=vals[:, 8 * r:8 * r + 8],
                                        in_values=cand_v)
                    if r < KR - 1:
                        nc.vector.match_replace(
                            out=cand_v,
                            in_to_replace=vals[:, 8 * r:8 * r + 8],
                            in_values=cand_v, imm_value=NEG_BIG)

                # phase 3 (gpsimd): invert rank->pos into ordered global ids.
                # rank_at[pos] = rank+1 (0 for non-winners); then scatter all
                # candidates to slot rank_at[c] -- non-winners pile up on
                # slot 0 (last-write-wins, verified on HW), winners land on
                # slots 1..56 in rank order.
                rank_at = sm_pool.tile([P, NSEG * 16], u16)
                nc.gpsimd.local_scatter(
                    out_ap=rank_at[:, :], data_ap=ranks1[:, :],
                    idxs_ap=wpos[:, :].bitcast(i16),
                    channels=P, num_elems=NSEG * 16, num_idxs=KR * 8)
                ordered = sm_pool.tile([P, 64], u16)
                nc.gpsimd.local_scatter(
                    out_ap=ordered[:, :], data_ap=cand_gi[:, :],
                    idxs_ap=rank_at[:, :].bitcast(i16),
                    channels=P, num_elems=64, num_idxs=NSEG * 16)
                nbr_sb = ordered[:, 1:]

            # dist = sqrt(relu(|p_i|^2 - s'))
            d2t = sm_pool.tile([P, K], f32)
            nc.scalar.activation(d2t, vals[:, :K], AF.Relu,
                                 bias=sq_part[:, t:t + 1], scale=-1.0)
            dist_t = out_pool.tile([P, K], f32)
            nc.scalar.activation(dist_t, d2t, AF.Sqrt)

            # RDF bins: exp(-(dist - c_b)^2 / (2 w^2))
            attr_t = out_pool.tile([P, K, NUM_BINS], f32)
            for b in range(NUM_BINS):
                u = sm_pool.tile([P, K], f32, tag="u_tmp")
                nc.scalar.activation(u, dist_t, AF.Square,
                                     bias=negc[:, b:b + 1])
                nc.scalar.activation(attr_t[:, :, b], u, AF.Exp,
                                     scale=-inv2w2)

            nc.sync.dma_start(out=nbr_d[ts(t, P), :], in_=nbr_sb[:, :K])
            nc.sync.dma_start(out=dist_d[ts(t, P), :], in_=dist_t)
            nc.sync.dma_start(
                out=attr_d[ts(t, P), :],
                in_=attr_t[:].rearrange("p a b -> p (a b)"),
            )

    if not nc.is_finalized():
        nc.finalize()
    return nc


def _get_nc():
    key = "nc_" + VARIANT
    if key not in _CACHE:
        _CACHE[key] = _build_bass(VARIANT)
    return _CACHE[key]


def run_device(pos_full, trace=False):
    """Run the SPMD kernel. Returns (per_core_results, BassKernelResults)."""
    from concourse.bass_utils import run_bass_kernel_spmd

    pos_full = np.ascontiguousarray(np.asarray(pos_full, dtype=np.float32))
    assert pos_full.shape == (N, 3)
    in_maps = [{"pos": pos_full[g * M:(g + 1) * M]} for g in range(G)]
    res = run_bass_kernel_spmd(_get_nc(), in_maps, list(range(G)), trace=trace)
    return res.results, res


_DST = None


def _static_dst():
    global _DST
    if _DST is None:
        _DST = np.repeat(np.arange(N, dtype=np.int32), K)
    return _DST


def assemble(results, pos=None):
    nbr = np.stack([r["nbr"].astype(np.int32, copy=False) for r in results])
    dist = np.stack([r["dist"] for r in results])        # [G, M, K]
    attr = np.stack([r["attr"] for r in results])        # [G, M, K*5]

    if pos is not None and "v16" in results[0]:
        # A 512-wide segment can (rarely) hold >16 of a row's top-50; such
        # rows are detectable: some segment's 16th-kept s' beats (or ties)
        # the row's 50th winner. Recompute those rows exactly on the host.
        v16 = np.stack([r["v16"] for r in results]).astype(np.float64)
        p64 = pos.astype(np.float64)
        sq = (p64 ** 2).sum(1).reshape(G, M)
        d2_50 = dist[:, :, K - 1].astype(np.float64) ** 2
        suspect = (v16 >= (sq - d2_50 - 1e-2)[:, :, None]).any(-1)
        centers = (np.arange(NUM_BINS) * (CUTOFF / (NUM_BINS - 1)))
        att4 = attr.reshape(G, M, K, NUM_BINS)
        for g, i in zip(*np.nonzero(suspect)):
            pg = p64[g * M:(g + 1) * M]
            d2 = ((pg - pg[i]) ** 2).sum(1)
            d2[i] = np.inf
            o = np.argsort(d2, kind="stable")[:K]
            nbr[g, i] = o
            dd = np.sqrt(d2[o])
            dist[g, i] = dd
            att4[g, i] = np.exp(-((dd[:, None] - centers) ** 2) / 12.5)

    offs = (np.arange(G, dtype=np.int32) * M)[:, None, None]
    src = (nbr + offs).reshape(-1)
    dst = _static_dst()

    E = N * K
    edge_index = np.empty((2, 2 * E), dtype=np.int32)
    edge_index[0, :E] = src
    edge_index[0, E:] = dst
    edge_index[1, :E] = dst
    edge_index[1, E:] = src

    d = dist.reshape(-1)
    dist_full = np.empty((2 * E, 1), dtype=np.float32)
    dist_full[:E, 0] = d
    dist_full[E:, 0] = d

    a = attr.reshape(-1, NUM_BINS)
    attr_full = np.empty((2 * E, NUM_BINS), dtype=np.float32)
    attr_full[:E] = a
    attr_full[E:] = a

    return edge_index, dist_full, attr_full


def _results_sane(results):
    """Cheap guard against rare transient device glitches (garbage runs)."""
    try:
        for r in results:
            if int(r["nbr"].max()) >= M:
                return False
            if not np.isfinite(r["dist"]).all():
                return False
    except Exception:
        return False
    return True


def kernel(pos, batch=None, num_graphs=None, **kw):
    pos = np.ascontiguousarray(np.asarray(pos, dtype=np.float32))
    results, _ = run_device(pos)
    if not _results_sane(results):
        results, _ = run_device(pos)
    return assemble(results, pos)


# revision 42
# speedup vs baseline: 1.1336x; 1.0082x over previous
"""KNN graph augmentation kernel for Trainium2 (8 NeuronCores, SPMD).

Problem: for 8 graphs of 4096 3-D points each, build the k=50 nearest
neighbor graph per graph (excluding self loops), then emit
  edge_index [2, 2*N*K]  (forward + reversed duplicate, int32)
  dist       [2*N*K, 1]  euclidean edge length (f32)
  edge_attr  [2*N*K, 5]  gaussian RDF expansion of dist (f32)

Sharding: data-parallel over graphs, one graph per NeuronCore.

Device algorithm per core (M=4096 nodes, 32 row-tiles of 128):
  - selection key s' = 2*p_i.p_j - |p_j|^2  (equals -d2 + |p_i|^2; the
    per-row constant does not affect per-row top-k order) via a K=4
    augmented fp32 matmul on the PE, evicted PSUM->SBUF by the scalar
    engine. Diagonal masked to -BIG with gpsimd affine_select.
  - top-50 per row ("opt" variant): exact top-16 of each 256-wide
    segment (DVE max8/find_index8/match_replace8, stage-batched), then
    a 256-candidate merge for the top-56 in order, then two gpsimd
    local_scatters invert rank->position into rank-ordered global
    column ids. A segment holding >16 of a row's top-50 would be the
    only error source; P ~ 3e-8 per segment-row (top-50 columns are
    uniform), ~0.02 expected rows over the whole problem.
  - dist = sqrt(relu(|p_i|^2 - s'_sel)), RDF bins on the scalar engine.
Host reassembles the full mirrored edge list (pure memcpy-class work).

Measured on trn2 (8 cores, SPMD): HW exec ~1.245 ms; vs jax CPU
reference L2 rel err ~5e-4 on edge_index (near-tie order flips only),
~1e-6 on dist/edge_attr.
"""

import numpy as np

N = 32768
G = 8
M = N // G          # 4096 nodes per graph/core
K = 50
NUM_BINS = 5
CUTOFF = 10.0
P = 128             # partitions
T = M // P          # 32 row tiles per core
CCH = 512           # matmul free-dim chunk
NCH = M // CCH      # 8 chunks
KR = 7              # ceil(K/8) rounds of 8-wide extraction
NEG_BIG = -3.0e38

# top-k variant: "safe" = 7x(max/max_index/match_replace) over full 4096 row;
# "opt" = per-256-segment top-16 + merge + gpsimd local_scatter index fixup
VARIANT = "opt"
NSEG = 8            # segments per row (opt variant)
W = M // NSEG       # 512 segment width

_CACHE = {}


def _build_bass(variant):
    import concourse.bacc as bacc
    import concourse.mybir as mybir
    from concourse.tile import TileContext
    from concourse.masks import make_identity
    from concourse.bass import ts
    from contextlib import ExitStack

    f32 = mybir.dt.float32
    u32 = mybir.dt.uint32
    u16 = mybir.dt.uint16
    i16 = mybir.dt.int16
    AF = mybir.ActivationFunctionType

    width = CUTOFF / (NUM_BINS - 1)          # 2.5
    inv2w2 = 1.0 / (2.0 * width * width)     # 0.08
    centers = [i * width for i in range(NUM_BINS)]

    nc = bacc.Bacc()
    pos_d = nc.dram_tensor("pos", [M, 3], f32, kind="ExternalInput")
    nbr_dt = u32 if variant == "safe" else u16
    nbr_d = nc.dram_tensor("nbr", [M, K], nbr_dt, kind="ExternalOutput")
    dist_d = nc.dram_tensor("dist", [M, K], f32, kind="ExternalOutput")
    attr_d = nc.dram_tensor("attr", [M, K * NUM_BINS], f32, kind="ExternalOutput")
    if variant == "opt":
        # 16th-kept s' per segment: lets the host detect (rare) capacity
        # overflow of a segment and recompute those rows exactly.
        v16_d = nc.dram_tensor("v16", [M, NSEG], f32, kind="ExternalOutput")

    with ExitStack() as ctx:
        tc = ctx.enter_context(TileContext(nc))
        const = ctx.enter_context(tc.tile_pool(name="const", bufs=1))
        setup = ctx.enter_context(tc.tile_pool(name="setup", bufs=3))
        setup_ps = ctx.enter_context(tc.tile_pool(name="setup_ps", bufs=3, space="PSUM"))
        sq_ps = ctx.enter_context(tc.tile_pool(name="sq_ps", bufs=1, space="PSUM"))
        mm_ps = ctx.enter_context(tc.tile_pool(name="mm_ps", bufs=4, space="PSUM"))
        s_pool = ctx.enter_context(tc.tile_pool(name="s_pool", bufs=4))
        sm_pool = ctx.enter_context(tc.tile_pool(name="sm_pool", bufs=6))
        out_pool = ctx.enter_context(tc.tile_pool(name="out_pool", bufs=6))

        identity = const.tile([P, P], f32)
        make_identity(nc, identity)

        lhsT = const.tile([4, M], f32)   # rows 0..2: p^T, row 3: ones
        rhs = const.tile([4, M], f32)    # rows 0..2: 2*p^T, row 3: -|p_j|^2
        ones3 = const.tile([3, 1], f32)
        sq_part = const.tile([P, T], f32)  # |p_i|^2 in row-tile layout
        # engine APs must start at partition 0: memset all 4 rows to 1.0,
        # rows 0..2 are overwritten by the transposes below; row 3 stays 1.
        nc.vector.memset(lhsT, 1.0)
        nc.vector.memset(ones3, 1.0)
        negc = const.tile([P, NUM_BINS], f32)  # per-bin bias = -center_b
        for b in range(NUM_BINS):
            nc.vector.memset(negc[:, b:b + 1], -centers[b])

        if variant == "opt":
            # candidate slot c = 16*seg + j  ->  global col = W*seg + local
            seg_off = const.tile([P, NSEG * 16], u16)
            nc.gpsimd.iota(seg_off, pattern=[[W, NSEG], [0, 16]], base=0,
                           channel_multiplier=0)
            ranks1 = const.tile([P, KR * 8], u16)    # 1..56
            nc.gpsimd.iota(ranks1, pattern=[[1, KR * 8]], base=1,
                           channel_multiplier=0)

        # --- setup: transpose pos into [3, M], accumulate |p|^2 ---
        for t in range(T):
            pos_t = setup.tile([P, 3], f32)
            nc.sync.dma_start(out=pos_t, in_=pos_d[t * P:(t + 1) * P, :])
            sq3 = setup.tile([P, 3], f32)
            nc.scalar.activation(sq3, pos_t, AF.Square,
                                 accum_out=sq_part[:, t:t + 1])
            ps_tr = setup_ps.tile([P, P], f32)
            nc.tensor.matmul(ps_tr[:3, :], pos_t, identity, is_transpose=True)
            nc.vector.tensor_copy(lhsT[0:3, ts(t, P)], ps_tr[:3, :])
            nc.scalar.activation(rhs[0:3, ts(t, P)], ps_tr[:3, :], AF.Copy,
                                 scale=2.0)

        # row 3 of rhs: -|p_j|^2 via ones.T @ (p^T * p^T), staged in a
        # partition-0 row then DMA'd to partition 3 (engine APs cannot
        # start at partition 3, DMA can).
        pT2 = const.tile([3, M], f32)
        negsq_row = const.tile([1, M], f32)
        for c in range(NCH):
            # per-chunk so chunk c only depends on transposes 4c..4c+3
            nc.vector.tensor_mul(pT2[:, ts(c, CCH)], lhsT[0:3, ts(c, CCH)],
                                 lhsT[0:3, ts(c, CCH)])
            ps_sq = sq_ps.tile([1, CCH], f32)
            nc.tensor.matmul(ps_sq, ones3, pT2[:, ts(c, CCH)])
            nc.scalar.activation(negsq_row[:, ts(c, CCH)], ps_sq, AF.Copy,
                                 scale=-1.0)
            nc.sync.dma_start(out=rhs[3:4, ts(c, CCH)],
                              in_=negsq_row[:, ts(c, CCH)])

        # --- main loop over 32 row tiles ---
        for t in range(T):
            s_tile = s_pool.tile([P, M], f32)
            for c in range(NCH):
                ps = mm_ps.tile([P, CCH], f32)
                nc.tensor.matmul(ps, lhsT[:, ts(t, P)], rhs[:, ts(c, CCH)],
                                 start=True, stop=True)
                nc.scalar.activation(s_tile[:, ts(c, CCH)], ps, AF.Copy)

            # mask self-distance: fill where (col - 128*t - p) == 0
            nc.gpsimd.affine_select(
                out=s_tile[:, ts(t, P)], in_=s_tile[:, ts(t, P)],
                compare_op=mybir.AluOpType.not_equal, fill=NEG_BIG,
                base=0, pattern=[[1, P]], channel_multiplier=-1,
            )

            if variant == "safe":
                vals = sm_pool.tile([P, KR * 8], f32)
                idxs = sm_pool.tile([P, KR * 8], u32)
                for r in range(KR):
                    nc.vector.max(out=vals[:, 8 * r:8 * r + 8], in_=s_tile)
                    nc.vector.max_index(out=idxs[:, 8 * r:8 * r + 8],
                                        in_max=vals[:, 8 * r:8 * r + 8],
                                        in_values=s_tile)
                    if r < KR - 1:
                        nc.vector.match_replace(
                            out=s_tile,
                            in_to_replace=vals[:, 8 * r:8 * r + 8],
                            in_values=s_tile, imm_value=NEG_BIG)
                nbr_sb = idxs
            else:
                # phase 1: top-16 of each 256-wide segment (values + local
                # idx). Emitted stage-batched: consecutive DVE ops are
                # independent (different segments), so the serial
                # max->max_index->match_replace latency is hidden by the
                # engine's in-order pipeline instead of paid per segment.
                cand_v = sm_pool.tile([P, NSEG * 16], f32)
                cand_li = sm_pool.tile([P, NSEG * 16], u16)

                def _seg(g):
                    return s_tile[:, g * W:(g + 1) * W]

                for g in range(NSEG):
                    nc.vector.max(out=cand_v[:, 16 * g:16 * g + 8],
                                  in_=_seg(g))
                for g in range(NSEG):
                    nc.vector.max_index(out=cand_li[:, 16 * g:16 * g + 8],
                                        in_max=cand_v[:, 16 * g:16 * g + 8],
                                        in_values=_seg(g))
                for g in range(NSEG):
                    nc.vector.match_replace(
                        out=_seg(g),
                        in_to_replace=cand_v[:, 16 * g:16 * g + 8],
                        in_values=_seg(g), imm_value=NEG_BIG)
                for g in range(NSEG):
                    nc.vector.max(out=cand_v[:, 16 * g + 8:16 * g + 16],
                                  in_=_seg(g))
                for g in range(NSEG):
                    nc.vector.max_index(
                        out=cand_li[:, 16 * g + 8:16 * g + 16],
                        in_max=cand_v[:, 16 * g + 8:16 * g + 16],
                        in_values=_seg(g))
                # global column id per candidate (u16 int ops: DVE only)
                cand_gi = sm_pool.tile([P, NSEG * 16], u16)
                nc.vector.tensor_add(cand_gi, cand_li, seg_off)

                # export each segment's 16th-kept value (overflow detector);
                # DMA reads the strided slots before the merge overwrites them
                nc.sync.dma_start(
                    out=v16_d[ts(t, P), :],
                    in_=cand_v[:].rearrange("p (s c) -> p s c", c=16)[:, :, 15],
                )

                # phase 2: merge 256 candidates -> top-56 values + positions
                vals = sm_pool.tile([P, KR * 8], f32)
                wpos = sm_pool.tile([P, KR * 8], u16)
                for r in range(KR):
                    nc.vector.max(out=vals[:, 8 * r:8 * r + 8], in_=cand_v)
                    nc.vector.max_index(out=wpos[:, 8 * r:8 * r + 8],
                                        in_max=vals[:, 8 * r:8 * r + 8],
                                        in_values=cand_v)
                    if r < KR - 1:
                        nc.vector.match_replace(
                            out=cand_v,
                            in_to_replace=vals[:, 8 * r:8 * r + 8],
                            in_values=cand_v, imm_value=NEG_BIG)

                # phase 3 (gpsimd): invert rank->pos into ordered global ids.
                # rank_at[pos] = rank+1 (0 for non-winners); then scatter all
                # candidates to slot rank_at[c] -- non-winners pile up on
                # slot 0 (last-write-wins, verified on HW), winners land on
                # slots 1..56 in rank order.
                rank_at = sm_pool.tile([P, NSEG * 16], u16)
                nc.gpsimd.local_scatter(
                    out_ap=rank_at[:, :], data_ap=ranks1[:, :],
                    idxs_ap=wpos[:, :].bitcast(i16),
                    channels=P, num_elems=NSEG * 16, num_idxs=KR * 8)
                ordered = sm_pool.tile([P, 64], u16)
                nc.gpsimd.local_scatter(
                    out_ap=ordered[:, :], data_ap=cand_gi[:, :],
                    idxs_ap=rank_at[:, :].bitcast(i16),
                    channels=P, num_elems=64, num_idxs=NSEG * 16)
                nbr_sb = ordered[:, 1:]

            # dist = sqrt(relu(|p_i|^2 - s'))
            d2t = sm_pool.tile([P, K], f32)
            nc.scalar.activation(d2t, vals[:, :K], AF.Relu,
                                 bias=sq_part[:, t:t + 1], scale=-1.0)
            dist_t = out_pool.tile([P, K], f32)
            nc.scalar.activation(dist_t, d2t, AF.Sqrt)

            # RDF bins: exp(-(dist - c_b)^2 / (2 w^2))
            attr_t = out_pool.tile([P, K, NUM_BINS], f32)
            for b in range(NUM_BINS):
                u = sm_pool.tile([P, K], f32, tag="u_tmp")
                nc.scalar.activation(u, dist_t, AF.Square,
                                     bias=negc[:, b:b + 1])
                nc.scalar.activation(attr_t[:, :, b], u, AF.Exp,
                                     scale=-inv2w2)

            nc.sync.dma_start(out=nbr_d[ts(t, P), :], in_=nbr_sb[:, :K])
            nc.sync.dma_start(out=dist_d[ts(t, P), :], in_=dist_t)
            nc.sync.dma_start(
                out=attr_d[ts(t, P), :],
                in_=attr_t[:].rearrange("p a b -> p (a b)"),
            )

    if not nc.is_finalized():
        nc.finalize()
    return nc


def _get_nc():
    key = "nc_" + VARIANT
    if key not in _CACHE:
        _CACHE[key] = _build_bass(VARIANT)
    return _CACHE[key]


def run_device(pos_full, trace=False):
    """Run the SPMD kernel. Returns (per_core_results, BassKernelResults)."""
    from concourse.bass_utils import run_bass_kernel_spmd

    pos_full = np.ascontiguousarray(np.asarray(pos_full, dtype=np.float32))
    assert pos_full.shape == (N, 3)
    in_maps = [{"pos": pos_full[g * M:(g + 1) * M]} for g in range(G)]
    res = run_bass_kernel_spmd(_get_nc(), in_maps, list(range(G)), trace=trace)
    return res.results, res


_DST = None


def _static_dst():
    global _DST
    if _DST is None:
        _DST = np.repeat(np.arange(N, dtype=np.int32), K)
    return _DST


def assemble(results, pos=None):
    nbr = np.stack([r["nbr"].astype(np.int32, copy=False) for r in results])
    dist = np.stack([r["dist"] for r in results])        # [G, M, K]
    attr = np.stack([r["attr"] for r in results])        # [G, M, K*5]

    if pos is not None and "v16" in results[0]:
        # A 512-wide segment can (rarely) hold >16 of a row's top-50; such
        # rows are detectable: some segment's 16th-kept s' beats (or ties)
        # the row's 50th winner. Recompute those rows exactly on the host.
        v16 = np.stack([r["v16"] for r in results]).astype(np.float64)
        p64 = pos.astype(np.float64)
        sq = (p64 ** 2).sum(1).reshape(G, M)
        d2_50 = dist[:, :, K - 1].astype(np.float64) ** 2
        suspect = (v16 >= (sq - d2_50 - 1e-2)[:, :, None]).any(-1)
        centers = (np.arange(NUM_BINS) * (CUTOFF / (NUM_BINS - 1)))
        att4 = attr.reshape(G, M, K, NUM_BINS)
        for g, i in zip(*np.nonzero(suspect)):
            pg = p64[g * M:(g + 1) * M]
            d2 = ((pg - pg[i]) ** 2).sum(1)
            d2[i] = np.inf
            o = np.argsort(d2, kind="stable")[:K]
            nbr[g, i] = o
            dd = np.sqrt(d2[o])
            dist[g, i] = dd
            att4[g, i] = np.exp(-((dd[:, None] - centers) ** 2) / 12.5)

    offs = (np.arange(G, dtype=np.int32) * M)[:, None, None]
    src = (nbr + offs).reshape(-1)
    dst = _static_dst()

    E = N * K
    edge_index = np.empty((2, 2 * E), dtype=np.int32)
    edge_index[0, :E] = src
    edge_index[0, E:] = dst
    edge_index[1, :E] = dst
    edge_index[1, E:] = src

    d = dist.reshape(-1)
    dist_full = np.empty((2 * E, 1), dtype=np.float32)
    dist_full[:E, 0] = d
    dist_full[E:, 0] = d

    a = attr.reshape(-1, NUM_BINS)
    attr_full = np.empty((2 * E, NUM_BINS), dtype=np.float32)
    attr_full[:E] = a
    attr_full[E:] = a

    return edge_index, dist_full, attr_full


def _results_sane(results):
    """Cheap guard against rare transient device glitches (garbage runs)."""
    try:
        for r in results:
            if int(r["nbr"].max()) >= M:
                return False
            if not np.isfinite(r["dist"]).all():
                return False
    except Exception:
        return False
    return True


def kernel(pos, batch=None, num_graphs=None, **kw):
    pos = np.ascontiguousarray(np.asarray(pos, dtype=np.float32))
    results, _ = run_device(pos)
    if not _results_sane(results):
        results, _ = run_device(pos)
    return assemble(results, pos)
